# revision 6
# baseline (speedup 1.0000x reference)
"""MixHop GNN kernel for one TRN2 chip (8 NeuronCores), Bass/Tile.

Math (matches the reference exactly):
    row/col = edge_index with self loops appended
    deg[t]  = #edges with col==t          (host: integer bincount)
    dinv    = 1/sqrt(deg)
    prop(h) = D^-1/2 (A+I) D^-1/2 h
            -> z = dinv*h; y[t] = sum_{edges s->t} z[s] (self loop is an
               explicit edge); h_out = dinv*y
    h1 = prop(x); h2 = prop(h1)
    out = relu(concat(x@W0+b0, h1@W1+b1, h2@W2+b2)) @ Wout + bout

Layout: FEATURE-MAJOR. Core c owns targets [c*N/8, (c+1)*N/8), permuted
by ascending degree so each 128-target window has a homogeneous slot
count D. The z table lives in SBUF as [128 feats, node] f32, in lo/hi
halves of 25000 columns (ap_gather num_elems limit). Per (window, phase)
one ap_gather pulls the D-padded neighbor slots (pads hit a zeroed
column) for all 128 features at once, and one strided tensor_reduce
produces the 128 per-target sums. z0 = dinv*x is computed on host and
shipped as the two phase tables; z1 is exchanged with one f32 AllGather
(kept in permuted order -- prop-2 gather indices compose the
permutation, precomputed on host). The head is transpose-free off the
feature-major h tiles.
"""
import numpy as np
import ml_dtypes

N = 50000
F = 128
NCORE = 8
NPC = N // NCORE          # 6250 targets per core
WIN = 128
NWIN = (NPC + WIN - 1) // WIN      # 49
NPAD = NWIN * WIN                  # 6272
HALF = 25000                       # nodes per table phase
TBLC = HALF + 4                    # + zeroed pad columns
ZCOL = HALF                        # pad slots gather this (zero) column
PER_HOP = 64
OUT = 64
MAXIDX = 4096                      # max idxs per ap_gather call


def _wrap16(flat):
    return np.tile(np.asarray(flat, dtype=np.int16).reshape(-1, 16).T, (8, 1))


def _preprocess(edge_index):
    row = np.asarray(edge_index[0], dtype=np.int64)
    col = np.asarray(edge_index[1], dtype=np.int64)
    rows = np.concatenate([row, np.arange(N, dtype=np.int64)])
    cols = np.concatenate([col, np.arange(N, dtype=np.int64)])
    deg = np.bincount(cols, minlength=N).astype(np.float64)
    dinv = (1.0 / np.sqrt(deg)).astype(np.float32)

    orders, ranks, nbrs = [], [], []
    dmax = np.zeros((NWIN, 2), dtype=np.int64)
    for c in range(NCORE):
        lo, hi = c * NPC, (c + 1) * NPC
        sel = (cols >= lo) & (cols < hi)
        r, t = rows[sel], cols[sel] - lo
        dtot = np.bincount(t, minlength=NPC)
        order = np.argsort(dtot, kind="stable")      # ascending degree
        rank = np.empty(NPC, dtype=np.int64)
        rank[order] = np.arange(NPC)
        tp = rank[t]
        # bucket edges per (phase, permuted target); keep raw src ids
        o_ph = (r >= HALF).astype(np.int64)
        nbr = [[[] for _ in range(NPC)], [[] for _ in range(NPC)]]
        for src, tgt, h in zip(r, tp, o_ph):
            nbr[h][tgt].append(src)
        dm = np.zeros((NWIN, 2), dtype=np.int64)
        for hh in range(2):
            cnt = np.array([len(x) for x in nbr[hh]])
            for w in range(NWIN):
                seg = cnt[w * WIN:min((w + 1) * WIN, NPC)]
                dm[w, hh] = seg.max() if len(seg) else 0
        dmax = np.maximum(dmax, dm)
        orders.append(order)
        ranks.append(rank)
        nbrs.append(nbr)

    Ds = tuple((int(dmax[w, 0]), int(dmax[w, 1])) for w in range(NWIN))
    ncols = sum(WIN * (d0 + d1) // 16 for d0, d1 in Ds)

    # prop-2 gather position of raw node s (permuted z1 table layout)
    pos2 = np.empty(N, dtype=np.int64)
    for c in range(NCORE):
        pos2[c * NPC:(c + 1) * NPC] = c * NPC + ranks[c]
    # table-local column for each raw node, per prop
    # prop1: node s -> col s - h*HALF ; prop2: node s -> pos2[s] - h*HALF

    percore = []
    for c in range(NCORE):
        nbr = nbrs[c]
        idx16 = np.empty((128, 2 * ncols), dtype=np.int16)
        for prop in range(2):
            off = prop * ncols
            for w in range(NWIN):
                for hh in range(2):
                    D = Ds[w][hh]
                    if D == 0:
                        continue
                    blk = np.full((WIN, D), ZCOL, dtype=np.int64)
                    for j in range(WIN):
                        p = w * WIN + j
                        if p < NPC:
                            lst = nbr[hh][p]
                            if lst:
                                a = np.asarray(lst, dtype=np.int64)
                                if prop == 1:
                                    a = pos2[a]
                                blk[j, :len(a)] = a - hh * HALF
                    flat = blk.reshape(-1)
                    wcols = len(flat) // 16
                    idx16[:, off:off + wcols] = _wrap16(flat)
                    off += wcols
        dvt = np.zeros(NPAD, dtype=np.float16)
        dvt[:NPC] = dinv[c * NPC + orders[c]]
        percore.append({
            "idx16": np.ascontiguousarray(idx16),
            "order": orders[c],
            "dinv_t": np.ascontiguousarray(
                np.broadcast_to(dvt, (128, NPAD)).copy()),
        })
    return Ds, percore, dinv


def _build(Ds):
    import concourse.bass as bass  # noqa: F401
    import concourse.bacc as bacc
    import concourse.tile as tile
    import concourse.mybir as mybir

    dt = mybir.dt
    f32 = dt.float32
    f16 = dt.float16
    bf16 = dt.bfloat16
    AF = mybir.ActivationFunctionType
    ALU = mybir.AluOpType

    icols = [WIN * (d0 + d1) // 16 for d0, d1 in Ds]
    IOFF = np.concatenate([[0], np.cumsum(icols)]).astype(int)
    NC1 = int(IOFF[-1])                     # idx cols per prop

    nc = bacc.Bacc("TRN2", target_bir_lowering=False, debug=False,
                   num_devices=NCORE)

    z0lo_in = nc.dram_tensor("z0lo", [128, TBLC], f32, kind="ExternalInput")
    z0hi_in = nc.dram_tensor("z0hi", [128, TBLC], f32, kind="ExternalInput")
    idx_in = nc.dram_tensor("idx16", [128, 2 * NC1], dt.int16,
                            kind="ExternalInput")
    xt_in = nc.dram_tensor("xt", [128, NPAD], bf16, kind="ExternalInput")
    dv_in = nc.dram_tensor("dinv_t", [128, NPAD], f16, kind="ExternalInput")
    w_in = [nc.dram_tensor(f"w{k}", [F, PER_HOP], bf16,
                           kind="ExternalInput") for k in range(3)]
    wo_in = nc.dram_tensor("wout", [3 * PER_HOP, OUT], bf16,
                           kind="ExternalInput")
    b_in = [nc.dram_tensor(f"b{k}", [PER_HOP, 1], f32,
                           kind="ExternalInput") for k in range(3)]
    bo_in = nc.dram_tensor("bout", [OUT, 1], f32, kind="ExternalInput")
    out_t = nc.dram_tensor("out_t", [OUT, NPC], f32, kind="ExternalOutput")

    z1b = nc.dram_tensor("z1b", [128, NPC], f32)
    z1f = nc.dram_tensor("z1f", [128 * NCORE, NPC], f32, addr_space="Shared")

    def ws(w):
        return slice(w * WIN, (w + 1) * WIN)

    with tile.TileContext(nc) as tc:
        with (
            tc.tile_pool(name="persist", bufs=1) as pp,
            tc.tile_pool(name="idxp", bufs=1) as ip,
            tc.tile_pool(name="gout", bufs=2) as gp,
            tc.tile_pool(name="hx", bufs=2) as hp,
            tc.tile_pool(name="zst", bufs=1) as zp,
            tc.tile_pool(name="psum", bufs=2, space="PSUM") as ps,
        ):
            # ---- persistent ----
            table = pp.tile([128, TBLC], f32)
            nc.vector.memset(table[:, HALF:], 0.0)
            dinv_sb = pp.tile([128, NPAD], f16)
            nc.sync.dma_start(out=dinv_sb[:], in_=dv_in[:])
            y_acc = pp.tile([128, NPAD], f32)
            idx_sb = pp.tile([128, NC1], dt.int16)
            nc.sync.dma_start(out=idx_sb[:], in_=idx_in.ap()[:, 0:NC1])
            w_sb = []
            for k in range(3):
                t = pp.tile([F, PER_HOP], bf16, tag=f"w{k}")
                nc.sync.dma_start(out=t[:], in_=w_in[k][:])
                w_sb.append(t)
            wo_sb = []
            for k in range(3):
                t = pp.tile([PER_HOP, OUT], bf16, tag=f"wo{k}")
                nc.sync.dma_start(
                    out=t[:], in_=wo_in.ap()[k * PER_HOP:(k + 1) * PER_HOP, :])
                wo_sb.append(t)
            b_sb = []
            for k in range(3):
                t = pp.tile([PER_HOP, 1], f32, tag=f"b{k}")
                nc.sync.dma_start(out=t[:], in_=b_in[k][:])
                b_sb.append(t)
            bo_sb = pp.tile([OUT, 1], f32)
            nc.sync.dma_start(out=bo_sb[:], in_=bo_in[:])
            r1 = pp.tile([PER_HOP, NPAD], bf16)

            def load_table_z0(src):
                nc.sync.dma_start(out=table[:, 0:HALF],
                                  in_=src.ap()[:, 0:HALF])

            def load_table_z1(h):
                nc.sync.dma_start(
                    out=table[:, 0:HALF].rearrange("p (c n) -> p c n", n=NPC),
                    in_=z1f.ap()[4 * h * 128:4 * (h + 1) * 128, :].rearrange(
                        "(c p) n -> p c n", p=128),
                )

            def prop_phase(prop, h, first):
                for w in range(NWIN):
                    D = Ds[w][h]
                    if D == 0:
                        continue
                    nidx = WIN * D
                    coff = (int(IOFF[w])
                            + (0 if h == 0 else WIN * Ds[w][0] // 16))
                    wcols = nidx // 16
                    idxb = idx_sb[:, coff:coff + wcols]
                    g = gp.tile([128, nidx], f32, tag="g")
                    chunk = (MAXIDX // (16 * D)) * 16 * D
                    assert chunk > 0, f"D={D} too large"
                    done = 0
                    while done < nidx:
                        n = min(nidx - done, chunk)
                        nc.gpsimd.ap_gather(
                            out_ap=g[:, done:done + n].rearrange(
                                "p (n d) -> p n d", d=1),
                            in_ap=table[:].rearrange("p (n d) -> p n d", d=1),
                            idxs_ap=idxb[:, done // 16:(done + n) // 16],
                            channels=128, num_elems=TBLC, d=1, num_idxs=n)
                        done += n
                    if first:
                        nc.vector.tensor_reduce(
                            out=y_acc[:, ws(w)],
                            in_=g[:].rearrange("p (t d) -> p t d", d=D),
                            axis=mybir.AxisListType.X, op=ALU.add)
                    else:
                        red = gp.tile([128, WIN], f32, tag="red")
                        nc.vector.tensor_reduce(
                            out=red[:],
                            in_=g[:].rearrange("p (t d) -> p t d", d=D),
                            axis=mybir.AxisListType.X, op=ALU.add)
                        nc.vector.tensor_tensor(
                            out=y_acc[:, ws(w)], in0=y_acc[:, ws(w)],
                            in1=red[:], op=ALU.add)

            # ---- prop 1 (z0 tables from host) ----
            load_table_z0(z0lo_in)
            prop_phase(0, 0, first=True)
            load_table_z0(z0hi_in)
            prop_phase(0, 1, first=False)

            # z1 = dinv^2 * y -> bounce (permuted order) -> AllGather
            CH = 625
            for i in range(NPC // CH):
                sl = slice(i * CH, (i + 1) * CH)
                zs = zp.tile([128, CH], f32, tag="zs")
                nc.vector.tensor_tensor(out=zs[:], in0=y_acc[:, sl],
                                        in1=dinv_sb[:, sl], op=ALU.mult)
                zs2 = zp.tile([128, CH], f32, tag="zs2")
                nc.vector.tensor_tensor(out=zs2[:], in0=zs[:],
                                        in1=dinv_sb[:, sl], op=ALU.mult)
                nc.sync.dma_start(out=z1b.ap()[:, sl], in_=zs2[:])
            nc.gpsimd.collective_compute(
                "AllGather", ALU.bypass,
                replica_groups=[list(range(NCORE))],
                ins=[z1b[:]], outs=[z1f[:]])

            # overlap with AllGather: h1 head hop (DVE mult + PE + scalar)
            for w in range(NWIN):
                h1w = hp.tile([128, WIN], bf16, tag="h1w")
                nc.vector.tensor_tensor(out=h1w[:], in0=y_acc[:, ws(w)],
                                        in1=dinv_sb[:, ws(w)], op=ALU.mult)
                cps1 = ps.tile([PER_HOP, WIN], f32, tag="cps1")
                nc.tensor.matmul(out=cps1[:], lhsT=w_sb[1][:],
                                 rhs=h1w[:], start=True, stop=True)
                nc.scalar.activation(out=r1[:, ws(w)], in_=cps1[:],
                                     func=AF.Relu, bias=b_sb[1][:])
            # swap in the prop-2 gather indices (permuted table layout)
            nc.sync.dma_start(out=idx_sb[:], in_=idx_in.ap()[:, NC1:2 * NC1])

            # ---- prop 2 (z1 tables, permuted layout) ----
            load_table_z1(0)
            prop_phase(1, 0, first=True)
            load_table_z1(1)
            prop_phase(1, 1, first=False)

            # ---- finish head per window ----
            for w in range(NWIN):
                xw = hp.tile([128, WIN], bf16, tag="xw")
                nc.sync.dma_start(out=xw[:], in_=xt_in.ap()[:, ws(w)])
                cps = ps.tile([PER_HOP, WIN], f32, tag="cps")
                nc.tensor.matmul(out=cps[:], lhsT=w_sb[0][:], rhs=xw[:],
                                 start=True, stop=True)
                rx = hp.tile([PER_HOP, WIN], bf16, tag="rx")
                nc.scalar.activation(out=rx[:], in_=cps[:],
                                     func=AF.Relu, bias=b_sb[0][:])
                h2w = hp.tile([128, WIN], bf16, tag="h2w")
                nc.vector.tensor_tensor(out=h2w[:], in0=y_acc[:, ws(w)],
                                        in1=dinv_sb[:, ws(w)], op=ALU.mult)
                cps2 = ps.tile([PER_HOP, WIN], f32, tag="c2")
                nc.tensor.matmul(out=cps2[:], lhsT=w_sb[2][:], rhs=h2w[:],
                                 start=True, stop=True)
                r2 = hp.tile([PER_HOP, WIN], bf16, tag="r2")
                nc.scalar.activation(out=r2[:], in_=cps2[:],
                                     func=AF.Relu, bias=b_sb[2][:])
                ops = ps.tile([OUT, WIN], f32, tag="ops")
                nc.tensor.matmul(out=ops[:], lhsT=wo_sb[0][:],
                                 rhs=rx[:], start=True, stop=False)
                nc.tensor.matmul(out=ops[:], lhsT=wo_sb[1][:],
                                 rhs=r1[:, ws(w)], start=False, stop=False)
                nc.tensor.matmul(out=ops[:], lhsT=wo_sb[2][:],
                                 rhs=r2[:], start=False, stop=True)
                ow = hp.tile([OUT, WIN], f32, tag="ow")
                nc.scalar.activation(out=ow[:], in_=ops[:],
                                     func=AF.Identity, bias=bo_sb[:])
                lim = min(NPC, (w + 1) * WIN) - w * WIN
                nc.sync.dma_start(out=out_t.ap()[:, w * WIN:w * WIN + lim],
                                  in_=ow[:, 0:lim])

    nc.compile()
    return nc


_CACHE = {}


def _get_nc(Ds):
    if Ds not in _CACHE:
        _CACHE[Ds] = _build(Ds)
    return _CACHE[Ds]


def make_in_maps(x, percore, dinv, W0, b0, W1, b1, W2, b2, Wout, bout):
    x = np.asarray(x, dtype=np.float32)
    z0 = x * dinv[:, None]                      # [N, F] f32, host-exact
    z0lo = np.zeros((128, TBLC), dtype=np.float32)
    z0lo[:, :HALF] = z0[:HALF].T
    z0hi = np.zeros((128, TBLC), dtype=np.float32)
    z0hi[:, :N - HALF] = z0[HALF:].T
    common = {
        "z0lo": np.ascontiguousarray(z0lo),
        "z0hi": np.ascontiguousarray(z0hi),
        "w0": np.asarray(W0).astype(ml_dtypes.bfloat16),
        "w1": np.asarray(W1).astype(ml_dtypes.bfloat16),
        "w2": np.asarray(W2).astype(ml_dtypes.bfloat16),
        "wout": np.asarray(Wout).astype(ml_dtypes.bfloat16),
        "b0": np.asarray(b0, dtype=np.float32).reshape(PER_HOP, 1),
        "b1": np.asarray(b1, dtype=np.float32).reshape(PER_HOP, 1),
        "b2": np.asarray(b2, dtype=np.float32).reshape(PER_HOP, 1),
        "bout": np.asarray(bout, dtype=np.float32).reshape(OUT, 1),
    }
    in_maps = []
    for c in range(NCORE):
        pc = percore[c]
        xp = np.zeros((NPAD, F), dtype=np.float32)
        xp[:NPC] = x[c * NPC:(c + 1) * NPC][pc["order"]]
        m = dict(common)
        m["idx16"] = pc["idx16"]
        m["dinv_t"] = pc["dinv_t"]
        m["xt"] = np.ascontiguousarray(xp.T.astype(ml_dtypes.bfloat16))
        in_maps.append(m)
    return in_maps


def run(inputs, trace=False):
    from concourse.bass_utils import run_bass_kernel_spmd

    Ds, percore, dinv = _preprocess(np.asarray(inputs["edge_index"]))
    nc = _get_nc(Ds)
    in_maps = make_in_maps(
        inputs["x"], percore, dinv, inputs["W0"], inputs["b0"],
        inputs["W1"], inputs["b1"], inputs["W2"], inputs["b2"],
        inputs["Wout"], inputs["bout"])
    res = run_bass_kernel_spmd(nc, in_maps, core_ids=list(range(NCORE)),
                               trace=trace)
    out = np.empty((N, OUT), dtype=np.float32)
    for c in range(NCORE):
        o = np.asarray(res.results[c]["out_t"]).T    # [NPC, OUT] permuted
        out[c * NPC + percore[c]["order"]] = o
    return out, res


def kernel(x, edge_index, W0, b0, W1, b1, W2, b2, Wout, bout):
    out, _ = run({"x": x, "edge_index": edge_index, "W0": W0, "b0": b0,
                  "W1": W1, "b1": b1, "W2": W2, "b2": b2,
                  "Wout": Wout, "bout": bout})
    return out


# revision 9
# speedup vs baseline: 7.3240x; 7.3240x over previous
"""MixHop GNN kernel for one TRN2 chip (8 NeuronCores), Bass/Tile.

Math (matches the reference exactly):
    row/col = edge_index with self loops appended
    deg[t]  = #edges with col==t            (host: integer bincount)
    dinv    = 1/sqrt(deg)                   (device: sqrt + reciprocal)
    h1[t]   = dinv_t * (sum_{s->t} dinv_s * x_s  + dinv_t * x_t)
    h2[t]   = dinv_t * (sum_{s->t} dinv_s * h1_s + dinv_t * h1_t)
    out = relu(concat(x@W0+b0, h1@W1+b1, h2@W2+b2)) @ Wout + bout

Sharding: core c owns target nodes [c*N/8, (c+1)*N/8). Edges (self loops
excluded -- those enter via the z_stage add, since the needed value is
resident) are bucketed by target into windows of 128 consecutive
targets, split by source (< 32768 vs >=, the int16 limit of dma_gather),
each part padded to blocks of 128 (uniform across cores -> one SPMD
program). Per window: dma_gather pulls source rows straight from the
raw x table (hop 1) / the AllGathered h1 table (hop 2) -- the source-
side norm factor is folded into the selection matrix S built with ONE
fused DVE op: S = (iota == tl) * dinv_src,
and a PE matmul S.T @ G accumulates the scaled segment-sum in PSUM.
Gather descriptor generation is the bottleneck engine (GpSimd SWDGE,
~8.4 ns/idx on one queue), so consecutive gather calls alternate
between SWDGE queues 0/1, which overlap generation (~5.9 ns/idx).
Only ONE collective remains (AllGather of h1); the hop-1 table is the
raw x input. The dense head is interleaved into the hop-2 window loop.
"""
import numpy as np
import ml_dtypes

N = 50000
F = 128
NCORE = 8
NPC = N // NCORE          # 6250 nodes per core
WIN = 128                 # targets per window
NWIN = (NPC + WIN - 1) // WIN   # 49 (48 full + 1 partial of 106)
PER_HOP = 64
OUT = 64
SPLIT = 32768             # int16 index limit for dma_gather tables
MAXBLK = 8                # max 1024 idxs per dma_gather call
PAD_TL = 300.0            # dummy-edge tl: matches no iota value -> zero S row


def _chunks(nb):
    out = []
    while nb > 0:
        c = min(nb, MAXBLK)
        out.append(c)
        nb -= c
    return out


def _preprocess(edge_index):
    """Bucket edges by (core, target-window, source-half); pad uniformly.

    Returns (NBL, NBH, per_core list of dicts with idx16, tl_t, ds_t
    (dinv_src per slot), deg_t).
    """
    row = np.asarray(edge_index[0], dtype=np.int64)
    col = np.asarray(edge_index[1], dtype=np.int64)
    deg = (np.bincount(col, minlength=N) + 1).astype(np.float64)
    dinv = (1.0 / np.sqrt(deg)).astype(np.float32)

    cores = []
    for c in range(NCORE):
        lo, hi = c * NPC, (c + 1) * NPC
        sel = (col >= lo) & (col < hi)
        r = row[sel]
        t = col[sel] - lo
        order = np.argsort(t, kind="stable")
        r, t = r[order], t[order]
        is_lo = r < SPLIT
        parts = []
        for w in range(NWIN):
            wsel = (t // WIN) == w
            rl = r[wsel & is_lo]
            tl_l = (t[wsel & is_lo] % WIN).astype(np.float32)
            rh = r[wsel & ~is_lo]
            tl_h = (t[wsel & ~is_lo] % WIN).astype(np.float32)
            parts.append((rl, tl_l, rh, tl_h))
        cores.append(parts)

    NBLs = tuple(int((max(len(cores[c][w][0]) for c in range(NCORE)) + 127)
                     // 128) for w in range(NWIN))
    NBHs = tuple(int((max(len(cores[c][w][2]) for c in range(NCORE)) + 127)
                     // 128) for w in range(NWIN))
    IDXOFF = np.concatenate(
        [[0], np.cumsum([(nl + nh) * 8 for nl, nh in zip(NBLs, NBHs)])]
    ).astype(int)
    TLOFF = np.concatenate(
        [[0], np.cumsum([nl + nh for nl, nh in zip(NBLs, NBHs)])]).astype(int)
    out = []
    for ci, parts in enumerate(cores):
        idx16 = np.zeros((128, int(IDXOFF[-1])), dtype=np.int16)
        tl_t = np.full((128, int(TLOFF[-1])), PAD_TL, dtype=np.float32)
        ds_t = np.zeros((128, int(TLOFF[-1])), dtype=np.float32)
        for w, (rl, tl_l, rh, tl_h) in enumerate(parts):
            nbl_w, nbh_w = NBLs[w], NBHs[w]
            nbt_w = nbl_w + nbh_w
            lo_chunks, hi_chunks = _chunks(nbl_w), _chunks(nbh_w)
            ilo = np.zeros(nbl_w * 128, dtype=np.int64)
            ilo[:len(rl)] = rl
            ihi = np.zeros(nbh_w * 128, dtype=np.int64)
            ihi[:len(rh)] = rh - SPLIT
            # tl / dinv_src streams: [lo blocks..., hi blocks...]
            tw = np.full(nbt_w * 128, PAD_TL, dtype=np.float32)
            tw[:len(tl_l)] = tl_l
            tw[nbl_w * 128:nbl_w * 128 + len(tl_h)] = tl_h
            tl_t[:, TLOFF[w]:TLOFF[w + 1]] = tw.reshape(nbt_w, 128).T
            dw = np.zeros(nbt_w * 128, dtype=np.float32)
            dw[:len(rl)] = dinv[rl]
            dw[nbl_w * 128:nbl_w * 128 + len(rh)] = dinv[rh]
            ds_t[:, TLOFF[w]:TLOFF[w + 1]] = dw.reshape(nbt_w, 128).T
            # idx stream: per call, [16-wrap then replicate x8 partitions]
            cols = []
            for chunks, arr in ((lo_chunks, ilo), (hi_chunks, ihi)):
                off = 0
                for cb in chunks:
                    a = arr[off:off + cb * 128].astype(np.int16)
                    cols.append(np.tile(a.reshape(-1, 16).T, (8, 1)))
                    off += cb * 128
            if cols:
                idx16[:, IDXOFF[w]:IDXOFF[w + 1]] = np.concatenate(
                    cols, axis=1)
        lo = ci * NPC
        dpad = np.ones(NWIN * WIN, dtype=np.float32)
        dpad[:NPC] = deg[lo:lo + NPC]
        out.append({"idx16": np.ascontiguousarray(idx16),
                    "tl_t": np.ascontiguousarray(tl_t),
                    "ds_t": np.ascontiguousarray(ds_t),
                    "deg_t": np.ascontiguousarray(
                        dpad.reshape(NWIN, WIN).T.astype(np.float32))})
    return NBLs, NBHs, out


def _build(NBLs, NBHs):
    import concourse.bass as bass  # noqa: F401
    import concourse.bacc as bacc
    import concourse.tile as tile
    import concourse.mybir as mybir
    from concourse.masks import make_identity

    dt = mybir.dt
    f32 = dt.float32
    bf16 = dt.bfloat16
    AF = mybir.ActivationFunctionType
    ALU = mybir.AluOpType
    NBTs = [nl + nh for nl, nh in zip(NBLs, NBHs)]
    NBTMAX = max(NBTs)
    IDXOFF = np.concatenate(
        [[0], np.cumsum([nbt * 8 for nbt in NBTs])]).astype(int)
    TLOFF = np.concatenate([[0], np.cumsum(NBTs)]).astype(int)
    NFULL = (NWIN - 1) * WIN                 # 6144 rows in full windows
    NLAST = NPC - NFULL                      # 106 rows in the partial window

    nc = bacc.Bacc("TRN2", target_bir_lowering=False, debug=False,
                   num_devices=NCORE, num_swdge_queues=2)

    x_in = nc.dram_tensor("x_full", [N, F], bf16, kind="ExternalInput")
    xo_in = nc.dram_tensor("x_own", [NPC, F], f32, kind="ExternalInput")
    idx_in = nc.dram_tensor("idx16", [128, int(IDXOFF[-1])], dt.int16,
                            kind="ExternalInput")
    tl_in = nc.dram_tensor("tl_t", [128, int(TLOFF[-1])], f32,
                           kind="ExternalInput")
    ds_in = nc.dram_tensor("ds_t", [128, int(TLOFF[-1])], f32,
                           kind="ExternalInput")
    deg_in = nc.dram_tensor("deg_t", [128, NWIN], f32, kind="ExternalInput")
    iot_in = nc.dram_tensor("iot", [128, 128], bf16, kind="ExternalInput")
    w0_in = nc.dram_tensor("w0", [F, PER_HOP], f32, kind="ExternalInput")
    w1_in = nc.dram_tensor("w1", [F, PER_HOP], f32, kind="ExternalInput")
    w2_in = nc.dram_tensor("w2", [F, PER_HOP], f32, kind="ExternalInput")
    wo_in = nc.dram_tensor("wout", [3 * PER_HOP, OUT], f32,
                           kind="ExternalInput")
    b0_in = nc.dram_tensor("b0", [PER_HOP, 1], f32, kind="ExternalInput")
    b1_in = nc.dram_tensor("b1", [PER_HOP, 1], f32, kind="ExternalInput")
    b2_in = nc.dram_tensor("b2", [PER_HOP, 1], f32, kind="ExternalInput")
    bo_in = nc.dram_tensor("bout", [OUT, 1], f32, kind="ExternalInput")
    out_t = nc.dram_tensor("out_t", [OUT, NPC], f32, kind="ExternalOutput")

    h1b = nc.dram_tensor("h1b", [NPC, F], bf16)
    h1f = nc.dram_tensor("h1f", [N, F], bf16, addr_space="Shared")

    def ts(w):
        return slice(w * WIN, (w + 1) * WIN)

    with tile.TileContext(nc) as tc:
        with (
            tc.tile_pool(name="persist", bufs=1) as pp,
            tc.tile_pool(name="gbuf", bufs=4) as gp,
            tc.tile_pool(name="work", bufs=4) as wp,
            tc.tile_pool(name="psum_y", bufs=2, space="PSUM") as psy,
            tc.tile_pool(name="psum_t", bufs=2, space="PSUM") as pst,
            tc.tile_pool(name="psum_d", bufs=2, space="PSUM") as psd,
        ):
            # ---- persistent loads ----
            idx_sb = pp.tile([128, int(IDXOFF[-1])], dt.int16)
            nc.sync.dma_start(out=idx_sb[:], in_=idx_in[:])
            tl_sb = pp.tile([128, int(TLOFF[-1])], f32)
            nc.sync.dma_start(out=tl_sb[:], in_=tl_in[:])
            ds_sb = pp.tile([128, int(TLOFF[-1])], f32)
            nc.sync.dma_start(out=ds_sb[:], in_=ds_in[:])
            iot_sb = pp.tile([128, 128], bf16)
            nc.sync.dma_start(out=iot_sb[:], in_=iot_in[:])
            deg_sb = pp.tile([128, NWIN], f32)
            nc.sync.dma_start(out=deg_sb[:], in_=deg_in[:])
            w0_sb = pp.tile([F, PER_HOP], f32)
            nc.sync.dma_start(out=w0_sb[:], in_=w0_in[:])
            w1_sb = pp.tile([F, PER_HOP], f32)
            nc.sync.dma_start(out=w1_sb[:], in_=w1_in[:])
            w2_sb = pp.tile([F, PER_HOP], f32)
            nc.sync.dma_start(out=w2_sb[:], in_=w2_in[:])
            wo_sb = []
            for k in range(3):
                t = pp.tile([PER_HOP, OUT], f32, tag=f"wo{k}")
                nc.sync.dma_start(
                    out=t[:], in_=wo_in.ap()[k * PER_HOP:(k + 1) * PER_HOP, :])
                wo_sb.append(t)
            b_sb = []
            for k, bin_ in enumerate((b0_in, b1_in, b2_in)):
                t = pp.tile([PER_HOP, 1], f32, tag=f"b{k}")
                nc.sync.dma_start(out=t[:], in_=bin_[:])
                b_sb.append(t)
            bo_sb = pp.tile([OUT, 1], f32)
            nc.sync.dma_start(out=bo_sb[:], in_=bo_in[:])
            ident = pp.tile([128, 128], f32)
            make_identity(nc, ident[:])

            # dinv = 1/sqrt(deg); dinv2 = dinv^2  (both [128, NWIN])
            sq = pp.tile([128, NWIN], f32)
            nc.scalar.activation(out=sq[:], in_=deg_sb[:], func=AF.Sqrt)
            dinv = pp.tile([128, NWIN], f32)
            nc.vector.reciprocal(out=dinv[:], in_=sq[:])
            dinv2 = pp.tile([128, NWIN], f32)
            nc.vector.tensor_tensor(out=dinv2[:], in0=dinv[:], in1=dinv[:],
                                    op=ALU.mult)

            # ---- load x (window-major: [p, w*128+f] = x[w*128+p, f]) ----
            x_sb = pp.tile([128, NWIN * WIN], f32)
            nc.vector.memset(x_sb[:, (NWIN - 1) * WIN:], 0.0)
            nc.sync.dma_start(
                out=x_sb[:].rearrange("p (w f) -> p w f", f=F)[:, 0:NWIN - 1, :],
                in_=xo_in.ap()[0:NFULL, :].rearrange("(w p) f -> p w f", p=128),
            )
            nc.sync.dma_start(
                out=x_sb[0:NLAST, (NWIN - 1) * WIN:],
                in_=xo_in.ap()[NFULL:NPC, :],
            )

            # z_stage: hop1 self term dinv*x; overwritten to dinv^2*h1 later
            z_stage = pp.tile([128, NWIN * WIN], f32)
            for w in range(NWIN):
                nc.vector.tensor_scalar_mul(
                    out=z_stage[:, ts(w)], in0=x_sb[:, ts(w)],
                    scalar1=dinv[:, w:w + 1])

            h1_sb = pp.tile([128, NWIN * WIN], f32)
            qctr = [0]

            def head(w, h2_sb):
                relus = []
                for k, (h_sb, wk_sb) in enumerate(
                        ((x_sb, w0_sb), (h1_sb, w1_sb), (h2_sb, w2_sb))):
                    tp = pst.tile([128, 128], f32, tag="tp")
                    nc.tensor.transpose(out=tp[:], in_=h_sb[:, ts(w)],
                                        identity=ident[:])
                    hT = wp.tile([128, 128], f32, tag="hT")
                    nc.vector.tensor_copy(out=hT[:], in_=tp[:])
                    cps = psd.tile([PER_HOP, 128], f32, tag="cps")
                    nc.tensor.matmul(out=cps[:], lhsT=wk_sb[:], rhs=hT[:],
                                     start=True, stop=True)
                    rk = wp.tile([PER_HOP, 128], f32, tag=f"r{k}")
                    nc.scalar.activation(out=rk[:], in_=cps[:], func=AF.Relu,
                                         bias=b_sb[k][:])
                    relus.append(rk)
                ops = psd.tile([OUT, 128], f32, tag="ops")
                for k in range(3):
                    nc.tensor.matmul(out=ops[:], lhsT=wo_sb[k][:],
                                     rhs=relus[k][:],
                                     start=(k == 0), stop=(k == 2))
                ow = wp.tile([OUT, 128], f32, tag="ow")
                nc.scalar.activation(out=ow[:], in_=ops[:],
                                     func=AF.Identity, bias=bo_sb[:])
                lim = min(NPC, (w + 1) * WIN) - w * WIN
                nc.sync.dma_start(out=out_t.ap()[:, w * WIN:w * WIN + lim],
                                  in_=ow[:, 0:lim])

            def prop(table, dss, h_out, hop2):
                """One propagation sweep; hop2 also runs the head."""
                for w in range(NWIN):
                    NBT = NBTs[w]
                    g = gp.tile([128, NBTMAX * F], bf16, tag="g")
                    icol = int(IDXOFF[w])
                    blk = 0
                    for part, cbs in ((0, _chunks(NBLs[w])),
                                      (1, _chunks(NBHs[w]))):
                        src = (table.ap()[0:SPLIT, :] if part == 0
                               else table.ap()[SPLIT:N, :])
                        for cb in cbs:
                            nc.gpsimd.dma_gather(
                                out_ap=g[:, blk * F:(blk + cb) * F].rearrange(
                                    "p (b f) -> p b f", f=F),
                                in_ap=src,
                                idxs_ap=idx_sb[:, icol:icol + cb * 8],
                                num_idxs=cb * 128, num_idxs_reg=cb * 128,
                                elem_size=F, queue_num=qctr[0] % 2)
                            qctr[0] += 1
                            icol += cb * 8
                            blk += cb
                    ps = psy.tile([128, F], f32)
                    for j in range(NBT):
                        col = int(TLOFF[w]) + j
                        s = wp.tile([128, 128], bf16, tag="s")
                        nc.vector.tensor_scalar(
                            out=s[:], in0=iot_sb[:],
                            scalar1=tl_sb[:, col:col + 1],
                            scalar2=dss[:, col:col + 1],
                            op0=ALU.is_equal, op1=ALU.mult)
                        nc.tensor.matmul(
                            out=ps[:], lhsT=s[:], rhs=g[:, j * F:(j + 1) * F],
                            start=(j == 0), stop=(j == NBT - 1))
                    # self loop term (z_stage), then h = dinv * ya
                    ya = wp.tile([128, F], f32, tag="ya")
                    nc.vector.tensor_tensor(
                        out=ya[:], in0=ps[:], in1=z_stage[:, ts(w)],
                        op=ALU.add)
                    nc.vector.tensor_scalar_mul(
                        out=h_out[:, ts(w)], in0=ya[:],
                        scalar1=dinv[:, w:w + 1])
                    if not hop2:
                        # stage hop-2 self term dinv*h1 (prop applies the
                        # remaining dinv_t); bounce h1 window
                        nc.vector.tensor_scalar_mul(
                            out=z_stage[:, ts(w)], in0=h_out[:, ts(w)],
                            scalar1=dinv[:, w:w + 1])
                        hb = wp.tile([128, F], bf16, tag="hb")
                        nc.vector.tensor_copy(out=hb[:], in_=h_out[:, ts(w)])
                        lim = min(NPC, (w + 1) * WIN) - w * WIN
                        nc.sync.dma_start(
                            out=h1b.ap()[w * WIN:w * WIN + lim, :],
                            in_=hb[0:lim, :])
                    else:
                        head(w, h_out)

            prop(x_in, ds_sb, h1_sb, hop2=False)
            nc.gpsimd.collective_compute(
                "AllGather", ALU.bypass,
                replica_groups=[list(range(NCORE))],
                ins=[h1b[:]], outs=[h1f[:]])

            # hop 2: same per-edge norm dinv_src * dinv_tgt (h1 is raw)
            h2_sb = pp.tile([128, NWIN * WIN], f32)
            prop(h1f, ds_sb, h2_sb, hop2=True)

    nc.compile()
    return nc


_CACHE = {}


def _get_nc(NBLs, NBHs):
    key = (tuple(NBLs), tuple(NBHs))
    if key not in _CACHE:
        _CACHE[key] = _build(NBLs, NBHs)
    return _CACHE[key]


def make_in_maps(x, pc, W0, b0, W1, b1, W2, b2, Wout, bout):
    iot = np.broadcast_to(
        np.arange(128, dtype=np.float32), (128, 128)).astype(ml_dtypes.bfloat16)
    x = np.ascontiguousarray(np.asarray(x, dtype=np.float32))
    common = {
        "iot": iot,
        "x_full": np.ascontiguousarray(x.astype(ml_dtypes.bfloat16)),
        "w0": np.asarray(W0, dtype=np.float32),
        "w1": np.asarray(W1, dtype=np.float32),
        "w2": np.asarray(W2, dtype=np.float32),
        "wout": np.asarray(Wout, dtype=np.float32),
        "b0": np.asarray(b0, dtype=np.float32).reshape(PER_HOP, 1),
        "b1": np.asarray(b1, dtype=np.float32).reshape(PER_HOP, 1),
        "b2": np.asarray(b2, dtype=np.float32).reshape(PER_HOP, 1),
        "bout": np.asarray(bout, dtype=np.float32).reshape(OUT, 1),
    }
    in_maps = []
    for c in range(NCORE):
        m = dict(common)
        m.update(pc[c])
        m["x_own"] = np.ascontiguousarray(x[c * NPC:(c + 1) * NPC])
        in_maps.append(m)
    return in_maps


def run(inputs, trace=False):
    from concourse.bass_utils import run_bass_kernel_spmd

    NBL, NBH, pc = _preprocess(np.asarray(inputs["edge_index"]))
    nc = _get_nc(NBL, NBH)
    in_maps = make_in_maps(
        inputs["x"], pc, inputs["W0"], inputs["b0"], inputs["W1"],
        inputs["b1"], inputs["W2"], inputs["b2"], inputs["Wout"],
        inputs["bout"])
    res = run_bass_kernel_spmd(nc, in_maps, core_ids=list(range(NCORE)),
                               trace=trace)
    out = np.empty((N, OUT), dtype=np.float32)
    for c in range(NCORE):
        out[c * NPC:(c + 1) * NPC] = np.asarray(res.results[c]["out_t"]).T
    return out, res


def kernel(x, edge_index, W0, b0, W1, b1, W2, b2, Wout, bout):
    out, _ = run({"x": x, "edge_index": edge_index, "W0": W0, "b0": b0,
                  "W1": W1, "b1": b1, "W2": W2, "b2": b2,
                  "Wout": Wout, "bout": bout})
    return out


# revision 10
# speedup vs baseline: 9.9994x; 1.3653x over previous
"""MixHop GNN kernel for one TRN2 chip (8 NeuronCores), Bass/Tile.

Math (matches the reference exactly):
    row/col = edge_index with self loops appended
    deg[t]  = #edges with col==t            (host: integer bincount)
    dinv    = 1/sqrt(deg)                   (device: sqrt + reciprocal)
    h1[t]   = dinv_t * (sum_{s->t} dinv_s * x_s  + dinv_t * x_t)
    h2[t]   = dinv_t * (sum_{s->t} dinv_s * h1_s + dinv_t * h1_t)
    out = relu(concat(x@W0+b0, h1@W1+b1, h2@W2+b2)) @ Wout + bout

Sharding: core c owns target nodes [c*N/8, (c+1)*N/8). Edges (self loops
excluded -- those enter via the z_stage add, since the needed value is
resident) are bucketed by target into windows of 128 consecutive
targets, split by source (< 32768 vs >=, the int16 limit of dma_gather),
each part padded to blocks of 128 (uniform across cores -> one SPMD
program). Per window: dma_gather pulls source rows straight from the
raw x table (hop 1) / the AllGathered h1 table (hop 2) -- the source-
side norm factor is folded into the selection matrix S built with ONE
fused DVE op: S = (iota == tl) * dinv_src,
and a PE matmul S.T @ G accumulates the scaled segment-sum in PSUM.
Gather descriptor generation is the bottleneck engine (GpSimd SWDGE,
~8.4 ns/idx on one queue), so consecutive gather calls alternate
between SWDGE queues 0/1, which overlap generation (~5.9 ns/idx).
Only ONE collective remains (AllGather of h1); the hop-1 table is the
raw x input. The dense head is interleaved into the hop-2 window loop.
"""
import numpy as np
import ml_dtypes

N = 50000
F = 128
NCORE = 8
NPC = N // NCORE          # 6250 nodes per core
WIN = 128                 # targets per window
NWIN = (NPC + WIN - 1) // WIN   # 49 (48 full + 1 partial of 106)
PER_HOP = 64
OUT = 64
SPLIT = 32768             # int16 index limit for dma_gather tables
MAXBLK = 8                # max 1024 idxs per dma_gather call
PAD_TL = 300.0            # dummy-edge tl: matches no iota value -> zero S row


def _chunks(nb):
    if nb <= 0:
        return []
    k = (nb + MAXBLK - 1) // MAXBLK
    base, rem = divmod(nb, k)
    return [base + (1 if i < rem else 0) for i in range(k)]


def _preprocess(edge_index):
    """Bucket edges by (core, target-window, source-half); pad uniformly.

    Returns (NBL, NBH, per_core list of dicts with idx16, tl_t, ds_t
    (dinv_src per slot), deg_t).
    """
    row = np.asarray(edge_index[0], dtype=np.int64)
    col = np.asarray(edge_index[1], dtype=np.int64)
    deg = (np.bincount(col, minlength=N) + 1).astype(np.float64)
    dinv = (1.0 / np.sqrt(deg)).astype(np.float32)

    cores = []
    for c in range(NCORE):
        lo, hi = c * NPC, (c + 1) * NPC
        sel = (col >= lo) & (col < hi)
        r = row[sel]
        t = col[sel] - lo
        order = np.argsort(t, kind="stable")
        r, t = r[order], t[order]
        is_lo = r < SPLIT
        parts = []
        for w in range(NWIN):
            wsel = (t // WIN) == w
            rl = r[wsel & is_lo]
            tl_l = (t[wsel & is_lo] % WIN).astype(np.float32)
            rh = r[wsel & ~is_lo]
            tl_h = (t[wsel & ~is_lo] % WIN).astype(np.float32)
            parts.append((rl, tl_l, rh, tl_h))
        cores.append(parts)

    NBLs = tuple(int((max(len(cores[c][w][0]) for c in range(NCORE)) + 127)
                     // 128) for w in range(NWIN))
    NBHs = tuple(int((max(len(cores[c][w][2]) for c in range(NCORE)) + 127)
                     // 128) for w in range(NWIN))
    IDXOFF = np.concatenate(
        [[0], np.cumsum([(nl + nh) * 8 for nl, nh in zip(NBLs, NBHs)])]
    ).astype(int)
    TLOFF = np.concatenate(
        [[0], np.cumsum([nl + nh for nl, nh in zip(NBLs, NBHs)])]).astype(int)
    out = []
    for ci, parts in enumerate(cores):
        idx16 = np.zeros((128, int(IDXOFF[-1])), dtype=np.int16)
        tl_t = np.full((128, int(TLOFF[-1])), PAD_TL, dtype=np.float32)
        ds_t = np.zeros((128, int(TLOFF[-1])), dtype=np.float32)
        for w, (rl, tl_l, rh, tl_h) in enumerate(parts):
            nbl_w, nbh_w = NBLs[w], NBHs[w]
            nbt_w = nbl_w + nbh_w
            lo_chunks, hi_chunks = _chunks(nbl_w), _chunks(nbh_w)
            ilo = np.zeros(nbl_w * 128, dtype=np.int64)
            ilo[:len(rl)] = rl
            ihi = np.zeros(nbh_w * 128, dtype=np.int64)
            ihi[:len(rh)] = rh - SPLIT
            # tl / dinv_src streams: [lo blocks..., hi blocks...]
            tw = np.full(nbt_w * 128, PAD_TL, dtype=np.float32)
            tw[:len(tl_l)] = tl_l
            tw[nbl_w * 128:nbl_w * 128 + len(tl_h)] = tl_h
            tl_t[:, TLOFF[w]:TLOFF[w + 1]] = tw.reshape(nbt_w, 128).T
            dw = np.zeros(nbt_w * 128, dtype=np.float32)
            dw[:len(rl)] = dinv[rl]
            dw[nbl_w * 128:nbl_w * 128 + len(rh)] = dinv[rh]
            ds_t[:, TLOFF[w]:TLOFF[w + 1]] = dw.reshape(nbt_w, 128).T
            # idx stream: per call, [16-wrap then replicate x8 partitions]
            cols = []
            for chunks, arr in ((lo_chunks, ilo), (hi_chunks, ihi)):
                off = 0
                for cb in chunks:
                    a = arr[off:off + cb * 128].astype(np.int16)
                    cols.append(np.tile(a.reshape(-1, 16).T, (8, 1)))
                    off += cb * 128
            if cols:
                idx16[:, IDXOFF[w]:IDXOFF[w + 1]] = np.concatenate(
                    cols, axis=1)
        lo = ci * NPC
        dpad = np.ones(NWIN * WIN, dtype=np.float32)
        dpad[:NPC] = deg[lo:lo + NPC]
        out.append({"idx16": np.ascontiguousarray(idx16),
                    "tl_t": np.ascontiguousarray(tl_t),
                    "ds_t": np.ascontiguousarray(ds_t),
                    "deg_t": np.ascontiguousarray(
                        dpad.reshape(NWIN, WIN).T.astype(np.float32))})
    return NBLs, NBHs, out


def _build(NBLs, NBHs):
    import concourse.bass as bass  # noqa: F401
    import concourse.bacc as bacc
    import concourse.tile as tile
    import concourse.mybir as mybir
    from concourse.masks import make_identity

    dt = mybir.dt
    f32 = dt.float32
    bf16 = dt.bfloat16
    AF = mybir.ActivationFunctionType
    ALU = mybir.AluOpType
    NBTs = [nl + nh for nl, nh in zip(NBLs, NBHs)]
    NBTMAX = max(NBTs)
    IDXOFF = np.concatenate(
        [[0], np.cumsum([nbt * 8 for nbt in NBTs])]).astype(int)
    TLOFF = np.concatenate([[0], np.cumsum(NBTs)]).astype(int)
    NFULL = (NWIN - 1) * WIN                 # 6144 rows in full windows
    NLAST = NPC - NFULL                      # 106 rows in the partial window

    nc = bacc.Bacc("TRN2", target_bir_lowering=False, debug=False,
                   num_devices=NCORE, num_swdge_queues=2)

    x_in = nc.dram_tensor("x_full", [N, F], bf16, kind="ExternalInput")
    xo_in = nc.dram_tensor("x_own", [NPC, F], f32, kind="ExternalInput")
    idx_in = nc.dram_tensor("idx16", [128, int(IDXOFF[-1])], dt.int16,
                            kind="ExternalInput")
    tl_in = nc.dram_tensor("tl_t", [128, int(TLOFF[-1])], f32,
                           kind="ExternalInput")
    ds_in = nc.dram_tensor("ds_t", [128, int(TLOFF[-1])], f32,
                           kind="ExternalInput")
    deg_in = nc.dram_tensor("deg_t", [128, NWIN], f32, kind="ExternalInput")
    iot_in = nc.dram_tensor("iot", [128, 128], bf16, kind="ExternalInput")
    w0_in = nc.dram_tensor("w0", [F, PER_HOP], f32, kind="ExternalInput")
    w1_in = nc.dram_tensor("w1", [F, PER_HOP], f32, kind="ExternalInput")
    w2_in = nc.dram_tensor("w2", [F, PER_HOP], f32, kind="ExternalInput")
    wo_in = nc.dram_tensor("wout", [3 * PER_HOP, OUT], f32,
                           kind="ExternalInput")
    b0_in = nc.dram_tensor("b0", [PER_HOP, 1], f32, kind="ExternalInput")
    b1_in = nc.dram_tensor("b1", [PER_HOP, 1], f32, kind="ExternalInput")
    b2_in = nc.dram_tensor("b2", [PER_HOP, 1], f32, kind="ExternalInput")
    bo_in = nc.dram_tensor("bout", [OUT, 1], f32, kind="ExternalInput")
    out_t = nc.dram_tensor("out_t", [OUT, NPC], f32, kind="ExternalOutput")

    h1b = nc.dram_tensor("h1b", [NPC, F], bf16)
    h1f = nc.dram_tensor("h1f", [N, F], bf16, addr_space="Shared")

    def ts(w):
        return slice(w * WIN, (w + 1) * WIN)

    with tile.TileContext(nc) as tc:
        with (
            tc.tile_pool(name="persist", bufs=1) as pp,
            tc.tile_pool(name="gbuf", bufs=4) as gp,
            tc.tile_pool(name="work", bufs=4) as wp,
            tc.tile_pool(name="psum_y", bufs=2, space="PSUM") as psy,
            tc.tile_pool(name="psum_t", bufs=2, space="PSUM") as pst,
            tc.tile_pool(name="psum_d", bufs=2, space="PSUM") as psd,
        ):
            # ---- persistent loads ----
            idx_sb = pp.tile([128, int(IDXOFF[-1])], dt.int16)
            nc.sync.dma_start(out=idx_sb[:], in_=idx_in[:])
            tl_sb = pp.tile([128, int(TLOFF[-1])], f32)
            nc.sync.dma_start(out=tl_sb[:], in_=tl_in[:])
            ds_sb = pp.tile([128, int(TLOFF[-1])], f32)
            nc.sync.dma_start(out=ds_sb[:], in_=ds_in[:])
            iot_sb = pp.tile([128, 128], bf16)
            nc.sync.dma_start(out=iot_sb[:], in_=iot_in[:])
            deg_sb = pp.tile([128, NWIN], f32)
            nc.sync.dma_start(out=deg_sb[:], in_=deg_in[:])
            w0_sb = pp.tile([F, PER_HOP], f32)
            nc.sync.dma_start(out=w0_sb[:], in_=w0_in[:])
            w1_sb = pp.tile([F, PER_HOP], f32)
            nc.sync.dma_start(out=w1_sb[:], in_=w1_in[:])
            w2_sb = pp.tile([F, PER_HOP], f32)
            nc.sync.dma_start(out=w2_sb[:], in_=w2_in[:])
            wo_sb = []
            for k in range(3):
                t = pp.tile([PER_HOP, OUT], f32, tag=f"wo{k}")
                nc.sync.dma_start(
                    out=t[:], in_=wo_in.ap()[k * PER_HOP:(k + 1) * PER_HOP, :])
                wo_sb.append(t)
            b_sb = []
            for k, bin_ in enumerate((b0_in, b1_in, b2_in)):
                t = pp.tile([PER_HOP, 1], f32, tag=f"b{k}")
                nc.sync.dma_start(out=t[:], in_=bin_[:])
                b_sb.append(t)
            bo_sb = pp.tile([OUT, 1], f32)
            nc.sync.dma_start(out=bo_sb[:], in_=bo_in[:])
            ident = pp.tile([128, 128], f32)
            make_identity(nc, ident[:])
            iot_wide = pp.tile([128, NBTMAX * 128], bf16)
            for j in range(NBTMAX):
                nc.vector.tensor_copy(out=iot_wide[:, j * 128:(j + 1) * 128],
                                      in_=iot_sb[:])

            # dinv = 1/sqrt(deg); dinv2 = dinv^2  (both [128, NWIN])
            sq = pp.tile([128, NWIN], f32)
            nc.scalar.activation(out=sq[:], in_=deg_sb[:], func=AF.Sqrt)
            dinv = pp.tile([128, NWIN], f32)
            nc.vector.reciprocal(out=dinv[:], in_=sq[:])
            dinv2 = pp.tile([128, NWIN], f32)
            nc.vector.tensor_tensor(out=dinv2[:], in0=dinv[:], in1=dinv[:],
                                    op=ALU.mult)

            # ---- load x (window-major: [p, w*128+f] = x[w*128+p, f]) ----
            x_sb = pp.tile([128, NWIN * WIN], f32)
            nc.vector.memset(x_sb[:, (NWIN - 1) * WIN:], 0.0)
            nc.sync.dma_start(
                out=x_sb[:].rearrange("p (w f) -> p w f", f=F)[:, 0:NWIN - 1, :],
                in_=xo_in.ap()[0:NFULL, :].rearrange("(w p) f -> p w f", p=128),
            )
            nc.sync.dma_start(
                out=x_sb[0:NLAST, (NWIN - 1) * WIN:],
                in_=xo_in.ap()[NFULL:NPC, :],
            )

            # z_stage: hop1 self term dinv*x; overwritten to dinv^2*h1 later
            z_stage = pp.tile([128, NWIN * WIN], f32)
            for w in range(NWIN):
                nc.vector.tensor_scalar_mul(
                    out=z_stage[:, ts(w)], in0=x_sb[:, ts(w)],
                    scalar1=dinv[:, w:w + 1])

            h1_sb = pp.tile([128, NWIN * WIN], f32)
            qctr = [0]

            def head(w, h2_sb):
                relus = []
                for k, (h_sb, wk_sb) in enumerate(
                        ((x_sb, w0_sb), (h1_sb, w1_sb), (h2_sb, w2_sb))):
                    tp = pst.tile([128, 128], f32, tag="tp")
                    nc.tensor.transpose(out=tp[:], in_=h_sb[:, ts(w)],
                                        identity=ident[:])
                    hT = wp.tile([128, 128], f32, tag="hT")
                    nc.vector.tensor_copy(out=hT[:], in_=tp[:])
                    cps = psd.tile([PER_HOP, 128], f32, tag="cps")
                    nc.tensor.matmul(out=cps[:], lhsT=wk_sb[:], rhs=hT[:],
                                     start=True, stop=True)
                    rk = wp.tile([PER_HOP, 128], f32, tag=f"r{k}")
                    nc.scalar.activation(out=rk[:], in_=cps[:], func=AF.Relu,
                                         bias=b_sb[k][:])
                    relus.append(rk)
                ops = psd.tile([OUT, 128], f32, tag="ops")
                for k in range(3):
                    nc.tensor.matmul(out=ops[:], lhsT=wo_sb[k][:],
                                     rhs=relus[k][:],
                                     start=(k == 0), stop=(k == 2))
                ow = wp.tile([OUT, 128], f32, tag="ow")
                nc.scalar.activation(out=ow[:], in_=ops[:],
                                     func=AF.Identity, bias=bo_sb[:])
                lim = min(NPC, (w + 1) * WIN) - w * WIN
                nc.sync.dma_start(out=out_t.ap()[:, w * WIN:w * WIN + lim],
                                  in_=ow[:, 0:lim])

            def prop(table, dss, h_out, hop2):
                """One propagation sweep; hop2 also runs the head."""
                for w in range(NWIN):
                    NBT = NBTs[w]
                    g = gp.tile([128, NBTMAX * F], bf16, tag="g")
                    icol = int(IDXOFF[w])
                    blk = 0
                    for part, cbs in ((0, _chunks(NBLs[w])),
                                      (1, _chunks(NBHs[w]))):
                        src = (table.ap()[0:SPLIT, :] if part == 0
                               else table.ap()[SPLIT:N, :])
                        for cb in cbs:
                            nc.gpsimd.dma_gather(
                                out_ap=g[:, blk * F:(blk + cb) * F].rearrange(
                                    "p (b f) -> p b f", f=F),
                                in_ap=src,
                                idxs_ap=idx_sb[:, icol:icol + cb * 8],
                                num_idxs=cb * 128, num_idxs_reg=cb * 128,
                                elem_size=F, queue_num=qctr[0] % 2)
                            qctr[0] += 1
                            icol += cb * 8
                            blk += cb
                    ps = psy.tile([128, F], f32)
                    c0, c1 = int(TLOFF[w]), int(TLOFF[w + 1])
                    sw = wp.tile([128, NBTMAX * 128], bf16, tag="s")
                    nc.vector.tensor_tensor(
                        out=sw[:, 0:NBT * 128].rearrange(
                            "p (b t) -> p b t", t=128),
                        in0=iot_wide[:, 0:NBT * 128].rearrange(
                            "p (b t) -> p b t", t=128),
                        in1=tl_sb[:, c0:c1].rearrange(
                            "p (b o) -> p b o", o=1).broadcast_to(
                            [128, NBT, 128]),
                        op=ALU.is_equal)
                    nc.vector.tensor_tensor(
                        out=sw[:, 0:NBT * 128].rearrange(
                            "p (b t) -> p b t", t=128),
                        in0=sw[:, 0:NBT * 128].rearrange(
                            "p (b t) -> p b t", t=128),
                        in1=dss[:, c0:c1].rearrange(
                            "p (b o) -> p b o", o=1).broadcast_to(
                            [128, NBT, 128]),
                        op=ALU.mult)
                    for j in range(NBT):
                        nc.tensor.matmul(
                            out=ps[:], lhsT=sw[:, j * 128:(j + 1) * 128],
                            rhs=g[:, j * F:(j + 1) * F],
                            start=(j == 0), stop=(j == NBT - 1))
                    # self loop term (z_stage), then h = dinv * ya
                    ya = wp.tile([128, F], f32, tag="ya")
                    nc.vector.tensor_tensor(
                        out=ya[:], in0=ps[:], in1=z_stage[:, ts(w)],
                        op=ALU.add)
                    nc.vector.tensor_scalar_mul(
                        out=h_out[:, ts(w)], in0=ya[:],
                        scalar1=dinv[:, w:w + 1])
                    if not hop2:
                        # stage hop-2 self term dinv*h1 (prop applies the
                        # remaining dinv_t); bounce h1 window
                        nc.vector.tensor_scalar_mul(
                            out=z_stage[:, ts(w)], in0=h_out[:, ts(w)],
                            scalar1=dinv[:, w:w + 1])
                        hb = wp.tile([128, F], bf16, tag="hb")
                        nc.vector.tensor_copy(out=hb[:], in_=h_out[:, ts(w)])
                        lim = min(NPC, (w + 1) * WIN) - w * WIN
                        nc.sync.dma_start(
                            out=h1b.ap()[w * WIN:w * WIN + lim, :],
                            in_=hb[0:lim, :])
                    else:
                        head(w, h_out)

            prop(x_in, ds_sb, h1_sb, hop2=False)
            nc.gpsimd.collective_compute(
                "AllGather", ALU.bypass,
                replica_groups=[list(range(NCORE))],
                ins=[h1b[:]], outs=[h1f[:]])

            # hop 2: same per-edge norm dinv_src * dinv_tgt (h1 is raw)
            h2_sb = pp.tile([128, NWIN * WIN], f32)
            prop(h1f, ds_sb, h2_sb, hop2=True)

    nc.compile()
    return nc


_CACHE = {}


def _get_nc(NBLs, NBHs):
    key = (tuple(NBLs), tuple(NBHs))
    if key not in _CACHE:
        _CACHE[key] = _build(NBLs, NBHs)
    return _CACHE[key]


def make_in_maps(x, pc, W0, b0, W1, b1, W2, b2, Wout, bout):
    iot = np.broadcast_to(
        np.arange(128, dtype=np.float32), (128, 128)).astype(ml_dtypes.bfloat16)
    x = np.ascontiguousarray(np.asarray(x, dtype=np.float32))
    common = {
        "iot": iot,
        "x_full": np.ascontiguousarray(x.astype(ml_dtypes.bfloat16)),
        "w0": np.asarray(W0, dtype=np.float32),
        "w1": np.asarray(W1, dtype=np.float32),
        "w2": np.asarray(W2, dtype=np.float32),
        "wout": np.asarray(Wout, dtype=np.float32),
        "b0": np.asarray(b0, dtype=np.float32).reshape(PER_HOP, 1),
        "b1": np.asarray(b1, dtype=np.float32).reshape(PER_HOP, 1),
        "b2": np.asarray(b2, dtype=np.float32).reshape(PER_HOP, 1),
        "bout": np.asarray(bout, dtype=np.float32).reshape(OUT, 1),
    }
    in_maps = []
    for c in range(NCORE):
        m = dict(common)
        m.update(pc[c])
        m["x_own"] = np.ascontiguousarray(x[c * NPC:(c + 1) * NPC])
        in_maps.append(m)
    return in_maps


def run(inputs, trace=False):
    from concourse.bass_utils import run_bass_kernel_spmd

    NBL, NBH, pc = _preprocess(np.asarray(inputs["edge_index"]))
    nc = _get_nc(NBL, NBH)
    in_maps = make_in_maps(
        inputs["x"], pc, inputs["W0"], inputs["b0"], inputs["W1"],
        inputs["b1"], inputs["W2"], inputs["b2"], inputs["Wout"],
        inputs["bout"])
    res = run_bass_kernel_spmd(nc, in_maps, core_ids=list(range(NCORE)),
                               trace=trace)
    out = np.empty((N, OUT), dtype=np.float32)
    for c in range(NCORE):
        out[c * NPC:(c + 1) * NPC] = np.asarray(res.results[c]["out_t"]).T
    return out, res


def kernel(x, edge_index, W0, b0, W1, b1, W2, b2, Wout, bout):
    out, _ = run({"x": x, "edge_index": edge_index, "W0": W0, "b0": b0,
                  "W1": W1, "b1": b1, "W2": W2, "b2": b2,
                  "Wout": Wout, "bout": bout})
    return out
